# revision 1
# baseline (speedup 1.0000x reference)
"""Multi-head causal attention (B=2, S=2048, D=1024, H=16) on 8 trn2 cores.

Sharding: tensor-parallel over heads. Each core owns 2 heads: a 128-column
slice of w_q/w_k/w_v and the matching 128-row slice of w_o. Every core
computes a full [B*S, D] partial output; the host sums the 8 partials and
adds the bias.

Per-core kernel (all matmuls in float32r -> full PE rate), fully pipelined
over s-chunks of 512:
  Q-proj(chunk) -> attention non-diag ks-tiles (need only this chunk's Q +
  earlier chunks' K/V, so they overlap this chunk's own K/V projections and
  V transposes) -> K/V-proj + V-transpose(chunk) -> deferred out-proj of the
  previous chunk -> attention diagonal tail -> batched 2MB store, with Tile
  overlapping everything via subregion deps. Chunk input loads are per-k-tile
  DMAs on the SP ring; stores/weights ride the ACT ring.

  - QT/KT/VT [128, 4096] from w.T-tiles (stationary) x xT-chunks (moving).
    x is pre-transposed on the host so the contraction dim is on partitions.
  - V re-laid-out to [seq, d] via PE transpose; each head's V gets a ones
    column appended so the AV matmul's PSUM row 64 accumulates the softmax
    denominator for free (M=65, heads sequential).
  - scoresT[ks, qs] matmul pair packed into disjoint PE row-groups
    (contraction is 64); both land in one 2-bank PSUM tile so a single ACT
    exp (scale=1/8 folds the 1/sqrt(hd)) covers both heads, halving ACT's
    per-instruction overhead. No max-subtraction (scores provably small).
  - causal masking: gpsimd affine_select zeroes the invalid half of the 4
    diagonal-chunk ET tiles (exp first, zero after -- exact).
  - ctx eviction divides by the denominator row (DVE mult by gpsimd-broadcast
    reciprocal); head 1 is DMA-shifted to partitions 64..127.
  - out-proj ctxT.T @ w_o_c per chunk, PSUM rotated through the idle proj
    slots, evicted by DVE, streamed to DRAM while attention continues.
"""

import sys

sys.path.insert(0, "/opt/trn_rl_repo")

import numpy as np

import concourse.bass as bass
import concourse.mybir as mybir
import concourse.tile as tile
from concourse import bacc
from concourse.bass_utils import run_bass_kernel_spmd

B, S, D, H, HD = 2, 2048, 1024, 16, 64
BS = B * S                  # 4096 flattened rows
NCORES = 8
DC = D // NCORES            # 128 head-dims per core (2 heads)
P = 128                     # partitions
SC = 512                    # s-chunk (moving free dim)
NSC = BS // SC              # 8 s-chunks over the flattened rows
NKT = D // P                # 8 k-tiles for the projections
NQC = S // SC               # 4 q-chunks per batch
NST = BS // P               # 32 s-tiles of 128
SPB = S // P                # 16 s-tiles per batch

F32 = mybir.dt.float32
F32R = mybir.dt.float32r

LABELS = {}


def _lbl(bi, label):
    try:
        LABELS[bi.ins.name] = label
    except Exception:
        pass
    return bi


def _build_nc(phases=("proj", "attn", "oproj")):
    nc = bacc.Bacc(None, target_bir_lowering=False)

    xT = nc.dram_tensor("xT", [D, BS], F32R, kind="ExternalInput")
    wq = nc.dram_tensor("wq", [D, DC], F32R, kind="ExternalInput")
    wk = nc.dram_tensor("wk", [D, DC], F32R, kind="ExternalInput")
    wv = nc.dram_tensor("wv", [D, DC], F32R, kind="ExternalInput")
    wo = nc.dram_tensor("wo", [DC, D], F32R, kind="ExternalInput")
    ident_d = nc.dram_tensor("ident", [P, P], F32R, kind="ExternalInput")
    out = nc.dram_tensor("out", [BS, D], F32, kind="ExternalOutput")

    with tile.TileContext(nc) as tc:
        with (
            tc.tile_pool(name="big", bufs=1) as big,
            tc.tile_pool(name="xts", bufs=2) as xts,
            tc.tile_pool(name="ob", bufs=2) as obs,
            tc.tile_pool(name="et", bufs=5) as etp,
            tc.tile_pool(name="small", bufs=2) as small,
            tc.tile_pool(name="ps_a", bufs=2, space="PSUM") as ps_a,   # proj + oproj [128,512]
            tc.tile_pool(name="ps_b", bufs=2, space="PSUM") as ps_b,   # score pairs [128,2,512] + vtr
            tc.tile_pool(name="ps_c", bufs=1, space="PSUM") as ps_c,   # ctx pair [65,2,512]
        ):
            qt = big.tile([P, BS], F32R, tag="qt")
            kt = big.tile([P, BS], F32R, tag="kt")
            vt = big.tile([P, BS], F32R, tag="vt")
            ctxT = big.tile([P, BS], F32R, tag="ctxT")
            vone = big.tile([P, 2, NST, 65], F32R, tag="vone")
            wq_sb = big.tile([P, NKT, DC], F32R, tag="wq")
            wk_sb = big.tile([P, NKT, DC], F32R, tag="wk")
            wv_sb = big.tile([P, NKT, DC], F32R, tag="wv")
            wo_sb = big.tile([P, D], F32R, tag="wo")
            ident = big.tile([P, P], F32R, tag="ident")

            nc.scalar.dma_start(wq_sb[:], wq.rearrange("(t p) m -> p t m", p=P))
            nc.scalar.dma_start(wk_sb[:], wk.rearrange("(t p) m -> p t m", p=P))
            nc.scalar.dma_start(wv_sb[:], wv.rearrange("(t p) m -> p t m", p=P))
            nc.scalar.dma_start(wo_sb[:], wo[:])
            nc.scalar.dma_start(ident[:], ident_d[:])
            nc.gpsimd.memset(vone[:].bitcast(F32), 1.0)

            xT_r = xT.rearrange("(t p) s -> t p s", p=P)

            def do_proj_q(sc):
                """Input chunk DMA + Q projection for s-chunk sc."""
                cols = slice(sc * SC, (sc + 1) * SC)
                xt_t = xts.tile([P, NKT, SC], F32R, tag="xt")
                for kq in range(NKT):
                    nc.sync.dma_start(xt_t[:, kq:kq+1, :], xT_r[kq:kq+1, :, cols].transpose([1, 0, 2]))
                psp = ps_a.tile([P, SC], F32, tag="proj")
                for k in range(NKT):
                    _lbl(nc.tensor.matmul(psp[:], wq_sb[:, k, :], xt_t[:, k, :],
                                          start=(k == 0), stop=(k == NKT - 1)),
                         f"proj{sc}")
                nc.scalar.copy(qt[:, cols], psp[:])
                return xt_t

            def do_proj_kv(sc, xt_t):
                """K/V projections + V transpose for s-chunk sc."""
                cols = slice(sc * SC, (sc + 1) * SC)
                for w_sb, dst in ((wk_sb, kt), (wv_sb, vt)):
                    psp = ps_a.tile([P, SC], F32, tag="proj")
                    for k in range(NKT):
                        _lbl(nc.tensor.matmul(psp[:], w_sb[:, k, :], xt_t[:, k, :],
                                              start=(k == 0), stop=(k == NKT - 1)),
                             f"proj{sc}")
                    nc.scalar.copy(dst[:, cols], psp[:])
                # V transpose for the 4 s-tiles of this chunk
                for gg in range(4):
                    g = sc * 4 + gg
                    psT = ps_b.tile([P, 2, SC], F32R, tag="sc")
                    _lbl(nc.tensor.transpose(psT[:, 0, 0:P], vt[:, g * P:(g + 1) * P], ident[:]), f"vtr{sc}")
                    nc.scalar.copy(vone[:, 0, g, 0:64], psT[:, 0, 0:64])
                    nc.scalar.copy(vone[:, 1, g, 0:64], psT[:, 0, 64:128])

            def do_ks_tile(b, j, t, psc):
                nks = 4 * (j + 1)
                g = b * SPB + t
                kcols = slice(g * P, (g + 1) * P)
                diag = t >= nks - 4
                mi = t - (nks - 4) if diag else 0
                v0 = mi * P            # first possibly-valid qs column
                w0 = min(v0, 256)      # matmul restriction (keep N >= 256)
                qw = slice(b * S + j * SC + w0, b * S + (j + 1) * SC)
                ps_s = ps_b.tile([P, 2, SC], F32, tag="sc")
                for h in range(2):
                    hp = slice(h * 64, (h + 1) * 64)
                    _lbl(nc.tensor.matmul(
                        ps_s[:, h, w0:], kt[hp, kcols], qt[hp, qw],
                        start=True, stop=True, tile_position=(h * 64, 0),
                    ), f"score b{b}j{j}t{t}")
                et = etp.tile([P, 2, SC], F32R, tag="et")
                if v0 > 0:
                    nc.gpsimd.memset(et[:, :, 0:v0].bitcast(F32), 0.0)
                nc.scalar.activation(
                    et[:, :, v0:], ps_s[:, :, v0:],
                    mybir.ActivationFunctionType.Exp, scale=0.125,
                )
                if diag:
                    nc.gpsimd.affine_select(
                        out=et[:, :, v0:], in_=et[:, :, v0:],
                        compare_op=mybir.AluOpType.is_ge,
                        fill=0.0, base=0,
                        pattern=[[0, 2], [1, SC - v0]], channel_multiplier=-1,
                    )
                for h in range(2):
                    _lbl(nc.tensor.matmul(
                        psc[:, h, w0:], vone[:, h, g, :], et[:, h, w0:],
                        start=(t == 0), stop=(t == nks - 1),
                    ), f"av b{b}j{j}t{t}")

            def do_attn_head(b, j):
                """Non-diagonal ks-tiles of q-chunk (b, j): need only chunk j's Q
                plus previous chunks' K/V -- runs while chunk j's K/V project."""
                psc = ps_c.tile([65, 2, SC], F32, tag="ctx")
                for t in range(4 * (j + 1) - 4):
                    do_ks_tile(b, j, t, psc)
                return psc

            def do_attn_tail(b, j, psc):
                """Diagonal ks-tiles + eviction/normalization for q-chunk (b, j)."""
                qcols = slice(b * S + j * SC, b * S + (j + 1) * SC)
                nks = 4 * (j + 1)
                for t in range(nks - 4, nks):
                    do_ks_tile(b, j, t, psc)
                # fast raw eviction frees the PSUM slot; normalize afterwards
                tmp = small.tile([65, 2, SC], F32, tag="tmp")
                nc.scalar.copy(tmp[:], psc[:])
                for h in range(2):
                    rec = small.tile([1, SC], F32, tag="rec")
                    recb = small.tile([64, SC], F32, tag="recb")
                    nc.vector.reciprocal(rec[:], tmp[64:65, h, :])
                    nc.gpsimd.partition_broadcast(recb[:], rec[:])
                    if h == 0:
                        nc.vector.tensor_mul(ctxT[0:64, qcols], tmp[0:64, h, :], recb[:])
                    else:
                        stg = small.tile([64, SC], F32R, tag="stg")
                        nc.vector.tensor_mul(stg[:], tmp[0:64, h, :], recb[:])
                        nc.scalar.dma_start(ctxT[64:128, qcols], stg[:])

            def do_oproj_chunk(b, j):
                """Out-proj + batched 2MB store (ACT ring) for q-chunk (b, j)."""
                st0 = (b * S + j * SC) // P
                ob = obs.tile([P, 4, 2, SC], F32, tag="ob")
                out_view = out.rearrange("(g p) (j f) -> p g j f", p=P, j=2)
                for st4 in range(4):
                    st = st0 + st4
                    for jo in range(2):
                        pso = ps_a.tile([P, SC], F32, tag="proj")
                        _lbl(nc.tensor.matmul(
                            pso[:], ctxT[:, st * P:(st + 1) * P],
                            wo_sb[:, jo * SC:(jo + 1) * SC],
                            start=True, stop=True,
                        ), f"oproj b{b}j{j}st{st4}jo{jo}")
                        nc.vector.tensor_copy(ob[:, st4, jo, :], pso[:])
                nc.scalar.dma_start(out_view[:, st0:st0 + 4, :, :], ob[:])

            # pipeline over s-chunks. proj chunk sc unlocks attention chunk
            # (b, j) with b*NQC+j == sc. The chunk's non-diagonal ks-tiles only
            # need chunk sc's Q (emitted first) + previous chunks' K/V, so they
            # overlap chunk sc's own K/V projections and V transposes; the
            # diagonal tail follows. Out-proj of the previous attention chunk
            # is deferred past the next proj chunk so its PSUM-slot reuse never
            # stalls the critical path.
            pending_oproj = []
            for sc in range(NSC if "proj" in phases else 0):
                b, j = sc // NQC, sc % NQC
                xt_t = do_proj_q(sc)
                psc = do_attn_head(b, j) if "attn" in phases else None
                do_proj_kv(sc, xt_t)
                if pending_oproj and "oproj" in phases:
                    do_oproj_chunk(*pending_oproj.pop(0))
                if "attn" in phases:
                    do_attn_tail(b, j, psc)
                    pending_oproj.append((b, j))
            if "oproj" in phases:
                for bj in pending_oproj:
                    do_oproj_chunk(*bj)

    nc.compile()
    return nc


_NC_CACHE = None


def _get_nc():
    global _NC_CACHE
    if _NC_CACHE is None:
        _NC_CACHE = _build_nc()
    return _NC_CACHE


def kernel(x, w_q, w_k, w_v, w_o, b_o):
    x = np.asarray(x, dtype=np.float32)
    w_q = np.asarray(w_q, dtype=np.float32)
    w_k = np.asarray(w_k, dtype=np.float32)
    w_v = np.asarray(w_v, dtype=np.float32)
    w_o = np.asarray(w_o, dtype=np.float32)
    b_o = np.asarray(b_o, dtype=np.float32)

    xT = np.ascontiguousarray(x.reshape(BS, D).T)

    nc = _get_nc()
    in_maps = []
    for c in range(NCORES):
        cols = slice(c * DC, (c + 1) * DC)
        in_maps.append({
            "xT": xT,
            "ident": np.eye(P, dtype=np.float32),
            "wq": np.ascontiguousarray(w_q[:, cols]),
            "wk": np.ascontiguousarray(w_k[:, cols]),
            "wv": np.ascontiguousarray(w_v[:, cols]),
            "wo": np.ascontiguousarray(w_o[cols, :]),
        })

    res = None
    for attempt in range(3):
        try:
            res = run_bass_kernel_spmd(nc, in_maps, list(range(NCORES)))
            break
        except Exception:
            if attempt == 2:
                raise
            import time
            time.sleep(2.0)
    acc = res.results[0]["out"].astype(np.float32)
    for c in range(1, NCORES):
        acc = acc + res.results[c]["out"]
    acc = acc + b_o[None, :]
    return acc.reshape(B, S, D)



# revision 4
# speedup vs baseline: 1.1687x; 1.1687x over previous
"""Multi-head causal attention (B=2, S=2048, D=1024, H=16) on 8 trn2 cores.

Sharding: tensor-parallel over heads. Each core owns 2 heads: a 128-column
slice of w_q/w_k/w_v and the matching 128-row slice of w_o. Every core
computes a full [B*S, D] partial output in bf16; the host sums the 8 partials
in f32 and adds the bias.

All data bf16 (matmuls 1 cycle/row at any free size, half the DMA bytes of
f32); PSUM accumulation stays f32. Per-core schedule is a single software-
pipelined PE instruction stream:

  - chunk sc=(b,j) of 512 query rows: scores/exp/AV tiles are woven with
    "filler" matmuls (this chunk's V-projection, the previous chunk's
    out-projection, and the NEXT chunk's Q/K projections) so the PE never
    waits on the ACT exp latency (~1.2us per tile).
  - V is re-laid-out [seq, hd] by DMA-XBAR transposes (16x128 tiles, ~112ns
    per 128x128 block) straight into `vone`, whose 65th column of ones makes
    the AV matmul accumulate the softmax denominator for free.
  - causal masking: exp first, then a gpsimd affine_select zeroes only the
    128-wide boundary block of each diagonal tile.
  - chunk eviction: ACT copies raw ctx+denominator PSUM to SBUF (freeing the
    PSUM fast); DVE computes reciprocals and normalizes; head 1 reaches
    partitions 64..127 via a gpsimd SWDGE SBUF-to-SBUF shift.
  - DMA rings: input loads + V transposes on SP; weights on ACT (t=0 only,
    exp stream is never blocked); 2MB-chunk stores + h1 shifts on the gpsimd
    SWDGE ring.
  - the final chunk's normalize/out-proj/store is split into two 256-column
    halves to shorten the drain tail.
"""

import sys

sys.path.insert(0, "/opt/trn_rl_repo")

import numpy as np
import ml_dtypes

import concourse.bass as bass
import concourse.mybir as mybir
import concourse.tile as tile
from concourse import bacc
from concourse.bass_utils import run_bass_kernel_spmd

B, S, D, H, HD = 2, 2048, 1024, 16, 64
BS = B * S                  # 4096 flattened rows
NCORES = 8
DC = D // NCORES            # 128 head-dims per core (2 heads)
P = 128                     # partitions
SC = 512                    # s-chunk (moving free dim)
NSC = BS // SC              # 8 s-chunks over the flattened rows
NKT = D // P                # 8 k-tiles for the projections
NQC = S // SC               # 4 q-chunks per batch
NST = BS // P               # 32 s-tiles of 128
SPB = S // P                # 16 s-tiles per batch

F32 = mybir.dt.float32
BF16 = mybir.dt.bfloat16
EXP = mybir.ActivationFunctionType.Exp


def _build_nc():
    nc = bacc.Bacc(None, target_bir_lowering=False)

    xT = nc.dram_tensor("xT", [D, BS], BF16, kind="ExternalInput")
    wq = nc.dram_tensor("wq", [P, NKT, DC], BF16, kind="ExternalInput")
    wk = nc.dram_tensor("wk", [P, NKT, DC], BF16, kind="ExternalInput")
    wv = nc.dram_tensor("wv", [P, NKT, DC], BF16, kind="ExternalInput")
    wo = nc.dram_tensor("wo", [DC, D], BF16, kind="ExternalInput")
    out = nc.dram_tensor("out", [BS, D], BF16, kind="ExternalOutput")

    with tile.TileContext(nc) as tc:
        with (
            tc.tile_pool(name="big", bufs=1) as big,
            tc.tile_pool(name="xts", bufs=2) as xts,
            tc.tile_pool(name="ob", bufs=2) as obs,
            tc.tile_pool(name="et", bufs=5) as etp,
            tc.tile_pool(name="small", bufs=2) as small,
            tc.tile_pool(name="ps_p", bufs=2, space="PSUM") as ps_p,   # proj + oproj [128,512]
            tc.tile_pool(name="ps_s", bufs=2, space="PSUM") as ps_sp,  # score pairs [128,2,512]
            tc.tile_pool(name="ps_c", bufs=1, space="PSUM") as ps_cp,  # ctx pair [65,2,512]
        ):
            qt = big.tile([P, BS], BF16, tag="qt")
            kt = big.tile([P, BS], BF16, tag="kt")
            vt = big.tile([P, BS], BF16, tag="vt")
            ctxT = big.tile([P, BS], BF16, tag="ctxT")
            vone = big.tile([P, 2, NST, 65], BF16, tag="vone")
            wq_sb = big.tile([P, NKT, DC], BF16, tag="wq")
            wk_sb = big.tile([P, NKT, DC], BF16, tag="wk")
            wv_sb = big.tile([P, NKT, DC], BF16, tag="wv")
            wo_sb = big.tile([P, D], BF16, tag="wo")

            nc.scalar.dma_start(wq_sb[:], wq[:])
            nc.scalar.dma_start(wk_sb[:], wk[:])
            nc.scalar.dma_start(wv_sb[:], wv[:])
            nc.scalar.dma_start(wo_sb[:], wo[:])
            nc.gpsimd.memset(vone[:], 1.0)

            xT_r = xT.rearrange("(t p) s -> t p s", p=P)
            out_view = out.rearrange("(g p) (j f) -> p g j f", p=P, j=2)

            xt_tiles = {}
            psc_tiles = {}

            def prefetch(sc):
                t = xts.tile([P, NKT, SC], BF16, tag="xt", name="xt")
                cols = slice(sc * SC, (sc + 1) * SC)
                for k in range(NKT):
                    nc.sync.dma_start(
                        t[:, k:k + 1, :], xT_r[k:k + 1, :, cols].transpose([1, 0, 2])
                    )
                xt_tiles[sc] = t

            def proj_fillers(sc, w_sb, dst, post=None):
                """8 single-matmul closures; the last also evicts and runs post."""
                cols = slice(sc * SC, (sc + 1) * SC)
                box = {}

                def mk(k):
                    def f():
                        if k == 0:
                            box["ps"] = ps_p.tile([P, SC], F32, tag="pp", name="pp")
                        nc.tensor.matmul(
                            box["ps"][:], w_sb[:, k, :], xt_tiles[sc][:, k, :],
                            start=(k == 0), stop=(k == NKT - 1),
                        )
                        if k == NKT - 1:
                            nc.vector.tensor_copy(dst[:, cols], box["ps"][:])
                            if post is not None:
                                post()
                    return f

                return [mk(k) for k in range(NKT)]

            def vtr_post(sc):
                def post():
                    for gg in range(4):
                        g = sc * 4 + gg
                        nc.sync.dma_start_transpose(
                            vone[:, 0:2, g, 0:64], vt[:, g * P:(g + 1) * P]
                        )
                return post

            def oproj_fillers(pc):
                """Out-projection of chunk pc: 8 matmul closures with DVE
                evictions into ob staging; the last issues the SWDGE store."""
                st0 = pc * 4
                box = {}

                def mk(i):
                    st4, jo = divmod(i, 2)

                    def f():
                        if i == 0:
                            box["ob"] = obs.tile([P, 4, 2, SC], BF16, tag="ob", name="ob")
                        pso = ps_p.tile([P, SC], F32, tag="pp", name="pp")
                        nc.tensor.matmul(
                            pso[:], ctxT[:, (st0 + st4) * P:(st0 + st4 + 1) * P],
                            wo_sb[:, jo * SC:(jo + 1) * SC], start=True, stop=True,
                        )
                        nc.vector.tensor_copy(box["ob"][:, st4, jo, :], pso[:])
                        if i == 7:
                            nc.gpsimd.dma_start(
                                out_view[:, st0:st0 + 4, :, :], box["ob"][:]
                            )
                    return f

                return [mk(i) for i in range(8)]

            def emit_s(sc, b, j, t, state):
                """Score matmul pair + exp (+ causal select on diag tiles)."""
                nks = 4 * (j + 1)
                g = b * SPB + t
                kcols = slice(g * P, (g + 1) * P)
                diag = t >= nks - 4
                v0 = (t - (nks - 4)) * P if diag else 0
                qw = slice(sc * SC + v0, (sc + 1) * SC)
                pss = ps_sp.tile([P, 2, SC], F32, tag="sc", name="sc")
                for h in range(2):
                    hp = slice(h * 64, (h + 1) * 64)
                    nc.tensor.matmul(
                        pss[:, h, v0:], kt[hp, kcols], qt[hp, qw],
                        start=True, stop=True, tile_position=(h * 64, 0),
                    )
                et = etp.tile([P, 2, SC], BF16, tag="et", name="et")
                nc.scalar.activation(et[:, :, v0:], pss[:, :, v0:], EXP, scale=0.125)
                if diag:
                    nc.gpsimd.affine_select(
                        out=et[:, :, v0:v0 + P], in_=et[:, :, v0:v0 + P],
                        compare_op=mybir.AluOpType.is_ge,
                        fill=0.0, base=0,
                        pattern=[[0, 2], [1, P]], channel_multiplier=-1,
                    )
                state[t] = (et, v0, g)

            def emit_a(sc, j, t, state, psc):
                nks = 4 * (j + 1)
                et, v0, g = state.pop(t)
                for h in range(2):
                    nc.tensor.matmul(
                        psc[:, h, v0:], vone[:, h, g, :], et[:, h, v0:],
                        start=(t == 0), stop=(t == nks - 1),
                    )

            def emit_norm(pc, lo=0, hi=SC):
                """Evict + normalize chunk pc's raw ctx columns [lo:hi)."""
                cw = hi - lo
                ccols = slice(pc * SC + lo, pc * SC + hi)
                psc = psc_tiles[pc]
                tmp = small.tile([65, 2, cw], BF16, tag="tmp", name="tmp")
                nc.scalar.copy(tmp[:], psc[:, :, lo:hi])
                for h in range(2):
                    rec = small.tile([1, cw], BF16, tag="rec", name="rec")
                    with nc.allow_low_precision(reason="bf16 softmax denom, tol 2e-2"):
                        nc.vector.reciprocal(rec[:], tmp[64:65, h, :])
                    recb = small.tile([64, cw], BF16, tag="recb", name="recb")
                    nc.gpsimd.partition_broadcast(recb[:], rec[:])
                    if h == 0:
                        nc.vector.tensor_mul(ctxT[0:64, ccols], tmp[0:64, h, :], recb[:])
                    else:
                        stg = small.tile([64, cw], BF16, tag="stg", name="stg")
                        nc.vector.tensor_mul(stg[:], tmp[0:64, h, :], recb[:])
                        nc.gpsimd.dma_start(ctxT[64:128, ccols], stg[:])

            def emit_oproj_half(pc, half):
                st0 = pc * 4 + half * 2
                ob = obs.tile([P, 2, 2, SC], BF16, tag="obh", name="obh")
                for i in range(4):
                    st2, jo = divmod(i, 2)
                    pso = ps_p.tile([P, SC], F32, tag="pp", name="pp")
                    nc.tensor.matmul(
                        pso[:], ctxT[:, (st0 + st2) * P:(st0 + st2 + 1) * P],
                        wo_sb[:, jo * SC:(jo + 1) * SC], start=True, stop=True,
                    )
                    nc.vector.tensor_copy(ob[:, st2, jo, :], pso[:])
                nc.gpsimd.dma_start(out_view[:, st0:st0 + 2, :, :], ob[:])

            # ---- main pipeline over s-chunks ----
            fq = []
            for sc in range(NSC):
                b, j = divmod(sc, NQC)
                nks = 4 * (j + 1)
                if sc == 0:
                    prefetch(0)
                    prefetch(1)
                    for f in proj_fillers(0, wq_sb, qt):
                        f()
                    for f in proj_fillers(0, wk_sb, kt):
                        f()
                else:
                    if sc + 1 < NSC:
                        prefetch(sc + 1)
                    emit_norm(sc - 1)
                vp = proj_fillers(sc, wv_sb, vt, post=vtr_post(sc))
                if j == 0:
                    for f in vp:
                        f()
                else:
                    fq.extend(vp)
                if sc > 0:
                    fq.extend(oproj_fillers(sc - 1))
                if sc + 1 < NSC:
                    fq.extend(proj_fillers(sc + 1, wq_sb, qt))
                    fq.extend(proj_fillers(sc + 1, wk_sb, kt))

                psc = ps_cp.tile([65, 2, SC], F32, tag="ctx", name="ctx")
                psc_tiles[sc] = psc
                state = {}
                emit_s(sc, b, j, 0, state)
                for t in range(nks):
                    want = 2 if t >= nks - 4 else 1
                    if j == 0 and t == 0:
                        want = 3
                    for _ in range(want):
                        if fq:
                            fq.pop(0)()
                    if t + 1 < nks:
                        emit_s(sc, b, j, t + 1, state)
                    emit_a(sc, j, t, state, psc)
                while fq:
                    fq.pop(0)()

            # final chunk: normalize + out-project + store in two halves
            for half in (0, 1):
                emit_norm(NSC - 1, half * 256, (half + 1) * 256)
                emit_oproj_half(NSC - 1, half)

    nc.compile()
    return nc


_NC_CACHE = None


def _get_nc():
    global _NC_CACHE
    if _NC_CACHE is None:
        _NC_CACHE = _build_nc()
    return _NC_CACHE


def kernel(x, w_q, w_k, w_v, w_o, b_o):
    BF = ml_dtypes.bfloat16
    x = np.asarray(x, dtype=np.float32)
    w_q = np.asarray(w_q, dtype=np.float32)
    w_k = np.asarray(w_k, dtype=np.float32)
    w_v = np.asarray(w_v, dtype=np.float32)
    w_o = np.asarray(w_o, dtype=np.float32)
    b_o = np.asarray(b_o, dtype=np.float32)

    xT = np.ascontiguousarray(x.reshape(BS, D).T).astype(BF)

    def w_layout(w, cols):
        # [D, DC] -> [P, NKT, DC] with row t*128+p at [p, t]
        return np.ascontiguousarray(
            w[:, cols].reshape(NKT, P, DC).transpose(1, 0, 2)
        ).astype(BF)

    nc = _get_nc()
    in_maps = []
    for c in range(NCORES):
        cols = slice(c * DC, (c + 1) * DC)
        in_maps.append({
            "xT": xT,
            "wq": w_layout(w_q, cols),
            "wk": w_layout(w_k, cols),
            "wv": w_layout(w_v, cols),
            "wo": np.ascontiguousarray(w_o[cols, :]).astype(BF),
        })

    res = None
    for attempt in range(3):
        try:
            res = run_bass_kernel_spmd(nc, in_maps, list(range(NCORES)))
            break
        except Exception:
            if attempt == 2:
                raise
            import time
            time.sleep(2.0)
    acc = res.results[0]["out"].astype(np.float32)
    for c in range(1, NCORES):
        acc = acc + res.results[c]["out"].astype(np.float32)
    acc = acc + b_o[None, :]
    return acc.reshape(B, S, D)


# revision 9
# speedup vs baseline: 1.2695x; 1.0862x over previous
"""Multi-head causal attention (B=2, S=2048, D=1024, H=16) on 8 trn2 cores.

Sharding: tensor-parallel over heads. Each core owns 2 heads: a 128-column
slice of w_q/w_k/w_v and the matching 128-row slice of w_o. Every core
computes a full [B*S, D] partial output in bf16; the host sums the 8 partials
in f32 and adds the bias.

All data bf16 (matmuls 1 cycle/row at any free size, half the DMA bytes of
f32); PSUM accumulation stays f32. Per-core schedule is a single software-
pipelined PE instruction stream:

  - chunk sc=(b,j) of 512 query rows: scores/exp/AV tiles are woven with
    "filler" matmuls -- the previous chunk's out-projection and the NEXT
    chunk's Q/K/V projections, round-robin interleaved so consecutive
    fillers never contend for the same PSUM slot -- hiding the ACT exp
    latency (~1.2us per score tile) and keeping the PE at full p-state.
  - Q/K/V projections and the V XBAR transposes for chunk sc+1 all run
    during chunk sc, so nothing in a chunk waits on its own projections.
  - V is re-laid-out [seq, hd] by DMA-XBAR transposes straight into `vone`,
    whose extra column of ones makes the AV matmul accumulate the softmax
    denominator for free.
  - causal masking: exp first, then a gpsimd affine_select zeroes only the
    128-wide boundary block of each diagonal tile.
  - chunk eviction: ACT copies raw ctx+denominator PSUM to SBUF (freeing the
    PSUM fast); DVE computes reciprocals and normalizes, writing head 1
    directly to partitions 64..127 (engine APs carry partition offsets).
  - DMA rings: input loads + V transposes on SP; weights on ACT (t=0 only,
    the exp stream is never blocked); 2MB-chunk stores on the gpsimd SWDGE
    ring; the final two half-stores on SP (shorter latency chain).
  - the final chunk's normalize/out-proj/store is split into two 256-column
    halves to shorten the drain tail.
"""

import sys

sys.path.insert(0, "/opt/trn_rl_repo")

import numpy as np
import ml_dtypes

import concourse.bass as bass
import concourse.mybir as mybir
import concourse.tile as tile
from concourse import bacc
from concourse.bass_utils import run_bass_kernel_spmd

B, S, D, H, HD = 2, 2048, 1024, 16, 64
BS = B * S                  # 4096 flattened rows
NCORES = 8
DC = D // NCORES            # 128 head-dims per core (2 heads)
P = 128                     # partitions
SC = 512                    # s-chunk (moving free dim)
NSC = BS // SC              # 8 s-chunks over the flattened rows
NKT = D // P                # 8 k-tiles for the projections
NQC = S // SC               # 4 q-chunks per batch
NST = BS // P               # 32 s-tiles of 128
SPB = S // P                # 16 s-tiles per batch

F32 = mybir.dt.float32
BF16 = mybir.dt.bfloat16
DEBUG_DUMP = False
EXP = mybir.ActivationFunctionType.Exp

LABELS = {}


def _lbl(bi, label):
    try:
        LABELS[bi.ins.name] = label
    except Exception:
        pass
    return bi


def _rr(*groups):
    """Round-robin interleave lists (preserving each list's order)."""
    out = []
    idx = [0] * len(groups)
    while True:
        progressed = False
        for gi, g in enumerate(groups):
            if idx[gi] < len(g):
                out.append(g[idx[gi]])
                idx[gi] += 1
                progressed = True
        if not progressed:
            return out


def _build_nc():
    nc = bacc.Bacc(None, target_bir_lowering=False)

    xT = nc.dram_tensor("xT", [D, BS], BF16, kind="ExternalInput")
    wq = nc.dram_tensor("wq", [P, NKT, DC], BF16, kind="ExternalInput")
    wk = nc.dram_tensor("wk", [P, NKT, DC], BF16, kind="ExternalInput")
    wv = nc.dram_tensor("wv", [P, NKT, DC], BF16, kind="ExternalInput")
    wo = nc.dram_tensor("wo", [DC, D], BF16, kind="ExternalInput")
    out = nc.dram_tensor("out", [BS, D], BF16, kind="ExternalOutput")
    if DEBUG_DUMP:
        dbg_vone = nc.dram_tensor("dbg_vone", [P, NST, 160], BF16, kind="ExternalOutput")
        dbg_qt = nc.dram_tensor("dbg_qt", [P, BS], BF16, kind="ExternalOutput")
        dbg_kt = nc.dram_tensor("dbg_kt", [P, BS], BF16, kind="ExternalOutput")
        dbg_ctxT = nc.dram_tensor("dbg_ctxT", [P, BS], BF16, kind="ExternalOutput")

    with tile.TileContext(nc) as tc:
        with (
            tc.tile_pool(name="big", bufs=1) as big,
            tc.tile_pool(name="xts", bufs=2) as xts,
            tc.tile_pool(name="ob", bufs=2) as obs,
            tc.tile_pool(name="et", bufs=5) as etp,
            tc.tile_pool(name="small", bufs=2) as small,
            tc.tile_pool(name="ps_p", bufs=2, space="PSUM") as ps_p,   # proj + oproj [128,512]
            tc.tile_pool(name="ps_s", bufs=2, space="PSUM") as ps_sp,  # score pairs [128,2,512]
            tc.tile_pool(name="ps_c", bufs=1, space="PSUM") as ps_cp,  # ctx pair [65,2,512]
        ):
            qt = big.tile([P, BS], BF16, tag="qt")
            kt = big.tile([P, BS], BF16, tag="kt")
            vt = big.tile([P, BS], BF16, tag="vt")
            ctxT = big.tile([P, BS], BF16, tag="ctxT")
            vone = big.tile([P, NST, 160], BF16, tag="vone")
            wq_sb = big.tile([P, NKT, DC], BF16, tag="wq")
            wk_sb = big.tile([P, NKT, DC], BF16, tag="wk")
            wv_sb = big.tile([P, NKT, DC], BF16, tag="wv")
            wo_sb = big.tile([P, D], BF16, tag="wo")

            nc.scalar.dma_start(wq_sb[:], wq[:])
            nc.scalar.dma_start(wk_sb[:], wk[:])
            nc.scalar.dma_start(wv_sb[:], wv[:])
            nc.scalar.dma_start(wo_sb[:], wo[:])
            nc.gpsimd.memset(vone[:], 1.0)

            xT_r = xT.rearrange("(t p) s -> t p s", p=P)
            out_view = out.rearrange("(g p) (j f) -> p g j f", p=P, j=2)

            xt_tiles = {}
            psc_tiles = {}

            def prefetch(sc):
                t = xts.tile([P, NKT, SC], BF16, tag="xt", name="xt")
                cols = slice(sc * SC, (sc + 1) * SC)
                for k in range(NKT):
                    nc.sync.dma_start(
                        t[:, k:k + 1, :], xT_r[k:k + 1, :, cols].transpose([1, 0, 2])
                    )
                xt_tiles[sc] = t

            def proj_fillers(sc, w_sb, dst, post=None, tagc=""):
                """8 single-matmul closures; the last also evicts and runs post."""
                cols = slice(sc * SC, (sc + 1) * SC)
                box = {}

                def mk(k):
                    def f():
                        if k == 0:
                            box["ps"] = ps_p.tile([P, SC], F32, tag="pp", name="pp")
                        _lbl(nc.tensor.matmul(
                            box["ps"][:], w_sb[:, k, :], xt_tiles[sc][:, k, :],
                            start=(k == 0), stop=(k == NKT - 1),
                        ), f"proj{sc}.{tagc}.k{k}")
                        if k == NKT - 1:
                            nc.vector.tensor_copy(dst[:, cols], box["ps"][:])
                            if post is not None:
                                post()
                    return f

                return [mk(k) for k in range(NKT)]

            def vtr_post(sc):
                def post():
                    for gg in range(4):
                        g = sc * 4 + gg
                        nc.sync.dma_start_transpose(
                            vone[:, g, 0:128], vt[:, g * P:(g + 1) * P]
                        )
                return post

            def vp_fillers(sc):
                return proj_fillers(sc, wv_sb, vt, post=vtr_post(sc), tagc="v")

            def oproj_fillers(pc):
                """Out-projection of chunk pc: 8 matmul closures with DVE
                evictions into ob staging; the last issues the SWDGE store."""
                st0 = pc * 4
                box = {}

                def mk(i):
                    st4, jo = divmod(i, 2)

                    def f():
                        if i == 0:
                            box["ob"] = obs.tile([P, 4, 2, SC], BF16, tag="ob", name="ob")
                        pso = ps_p.tile([P, SC], F32, tag="pp", name="pp")
                        _lbl(nc.tensor.matmul(
                            pso[:], ctxT[:, (st0 + st4) * P:(st0 + st4 + 1) * P],
                            wo_sb[:, jo * SC:(jo + 1) * SC], start=True, stop=True,
                        ), f"oproj{pc}.{i}")
                        nc.vector.tensor_copy(box["ob"][:, st4, jo, :], pso[:])
                        if i == 7:
                            nc.gpsimd.dma_start(
                                out_view[:, st0:st0 + 4, :, :], box["ob"][:]
                            )
                    return f

                return [mk(i) for i in range(8)]

            def emit_s(sc, b, j, t, state):
                """Score matmul pair + exp (+ causal select on diag tiles)."""
                nks = 4 * (j + 1)
                g = b * SPB + t
                kcols = slice(g * P, (g + 1) * P)
                diag = t >= nks - 4
                v0 = (t - (nks - 4)) * P if diag else 0
                qw = slice(sc * SC + v0, (sc + 1) * SC)
                pss = ps_sp.tile([P, 2, SC], F32, tag="sc", name="sc")
                for h in range(2):
                    hp = slice(h * 64, (h + 1) * 64)
                    _lbl(nc.tensor.matmul(
                        pss[:, h, v0:], kt[hp, kcols], qt[hp, qw],
                        start=True, stop=True, tile_position=(h * 64, 0),
                    ), f"score{sc}.t{t}.h{h}")
                et = etp.tile([P, 2, SC], BF16, tag="et", name="et")
                _lbl(nc.scalar.activation(et[:, :, v0:], pss[:, :, v0:], EXP,
                                          scale=0.125), f"exp{sc}.t{t}")
                if diag:
                    nc.gpsimd.affine_select(
                        out=et[:, :, v0:v0 + P], in_=et[:, :, v0:v0 + P],
                        compare_op=mybir.AluOpType.is_ge,
                        fill=0.0, base=0,
                        pattern=[[0, 2], [1, P]], channel_multiplier=-1,
                    )
                state[t] = (et, v0, g)

            def emit_a(sc, j, t, state, psc):
                nks = 4 * (j + 1)
                et, v0, g = state.pop(t)
                for h in range(2):
                    _lbl(nc.tensor.matmul(
                        psc[:, h, v0:], vone[:, g, h:h + 129:2], et[:, h, v0:],
                        start=(t == 0), stop=(t == nks - 1),
                    ), f"av{sc}.t{t}.h{h}")

            def emit_norm(pc, lo=0, hi=SC):
                """Evict + normalize chunk pc's raw ctx columns [lo:hi)."""
                cw = hi - lo
                ccols = slice(pc * SC + lo, pc * SC + hi)
                psc = psc_tiles[pc]
                tmp = small.tile([65, 2, cw], BF16, tag="tmp", name="tmp")
                nc.scalar.copy(tmp[:], psc[:, :, lo:hi])
                for h in range(2):
                    rec = small.tile([1, cw], BF16, tag="rec", name="rec")
                    with nc.allow_low_precision(reason="bf16 softmax denom, tol 2e-2"):
                        nc.vector.reciprocal(rec[:], tmp[64:65, h, :])
                    recb = small.tile([64, cw], BF16, tag="recb", name="recb")
                    nc.gpsimd.partition_broadcast(recb[:], rec[:])
                    rows = slice(h * 64, h * 64 + 64)
                    nc.vector.tensor_mul(ctxT[rows, ccols], tmp[0:64, h, :], recb[:])

            def emit_oproj_half(pc, half):
                st0 = pc * 4 + half * 2
                ob = obs.tile([P, 2, 2, SC], BF16, tag="obh", name="obh")
                for i in range(4):
                    st2, jo = divmod(i, 2)
                    pso = ps_p.tile([P, SC], F32, tag="pp", name="pp")
                    _lbl(nc.tensor.matmul(
                        pso[:], ctxT[:, (st0 + st2) * P:(st0 + st2 + 1) * P],
                        wo_sb[:, jo * SC:(jo + 1) * SC], start=True, stop=True,
                    ), f"oprojh{pc}.{half}.{i}")
                    if i % 2 == 0:
                        nc.vector.tensor_copy(ob[:, st2, jo, :], pso[:])
                    else:
                        nc.scalar.copy(ob[:, st2, jo, :], pso[:])
                nc.sync.dma_start(out_view[:, st0:st0 + 2, :, :], ob[:])

            # ---- main pipeline over s-chunks ----
            for sc in range(NSC):
                b, j = divmod(sc, NQC)
                nks = 4 * (j + 1)
                if sc == 0:
                    prefetch(0)
                    prefetch(1)
                    for f in proj_fillers(0, wq_sb, qt, tagc="q"):
                        f()
                    for f in proj_fillers(0, wk_sb, kt, tagc="k"):
                        f()
                    for f in vp_fillers(0):
                        f()
                else:
                    if sc + 1 < NSC:
                        prefetch(sc + 1)
                    emit_norm(sc - 1)

                # fillers run during chunk sc: prev chunk's out-proj (deferred
                # one extra chunk near the end so the last chunk stays fed)
                # plus ALL of chunk sc+1's projections.
                groups = []
                if sc == NSC - 1:
                    groups.append(oproj_fillers(sc - 2))
                    groups.append(oproj_fillers(sc - 1))
                elif sc > 0:
                    if sc != NSC - 2:
                        groups.append(oproj_fillers(sc - 1))
                if sc + 1 < NSC:
                    groups.append(vp_fillers(sc + 1))
                    groups.append(proj_fillers(sc + 1, wq_sb, qt, tagc="q"))
                    groups.append(proj_fillers(sc + 1, wk_sb, kt, tagc="k"))
                fq = _rr(*groups)

                psc = ps_cp.tile([65, 2, SC], F32, tag="ctx", name="ctx")
                psc_tiles[sc] = psc
                state = {}
                emit_s(sc, b, j, 0, state)
                for t in range(nks):
                    want = 5 if (t >= nks - 4 or sc == 0) else 1
                    for _ in range(want):
                        if fq:
                            fq.pop(0)()
                    if t + 1 < nks:
                        emit_s(sc, b, j, t + 1, state)
                    emit_a(sc, j, t, state, psc)
                while fq:
                    fq.pop(0)()

            # final chunk: normalize + out-project + store in two halves
            for half in (0, 1):
                emit_norm(NSC - 1, half * 256, (half + 1) * 256)
                emit_oproj_half(NSC - 1, half)

            if DEBUG_DUMP:
                nc.sync.dma_start(dbg_vone[:], vone[:])
                nc.sync.dma_start(dbg_qt[:], qt[:])
                nc.sync.dma_start(dbg_kt[:], kt[:])
                nc.sync.dma_start(dbg_ctxT[:], ctxT[:])

    nc.compile()
    return nc


_NC_CACHE = None


def _get_nc():
    global _NC_CACHE
    if _NC_CACHE is None:
        _NC_CACHE = _build_nc()
    return _NC_CACHE


def kernel(x, w_q, w_k, w_v, w_o, b_o):
    BF = ml_dtypes.bfloat16
    x = np.asarray(x, dtype=np.float32)
    w_q = np.asarray(w_q, dtype=np.float32)
    w_k = np.asarray(w_k, dtype=np.float32)
    w_v = np.asarray(w_v, dtype=np.float32)
    w_o = np.asarray(w_o, dtype=np.float32)
    b_o = np.asarray(b_o, dtype=np.float32)

    xT = np.ascontiguousarray(x.reshape(BS, D).T).astype(BF)

    def w_layout(w, cols):
        # [D, DC] -> [P, NKT, DC] with row t*128+p at [p, t]
        return np.ascontiguousarray(
            w[:, cols].reshape(NKT, P, DC).transpose(1, 0, 2)
        ).astype(BF)

    # interleave V head-dims: projection row r holds head r%2, dim r//2, so
    # the plain [128,128] XBAR transpose lands v columns exactly where the AV
    # matmul's strided stationary slice reads them.
    vperm = np.array([(r % 2) * 64 + r // 2 for r in range(DC)])

    nc = _get_nc()
    in_maps = []
    for c in range(NCORES):
        cols = slice(c * DC, (c + 1) * DC)
        wv_c = np.ascontiguousarray(w_v[:, cols][:, vperm])
        in_maps.append({
            "xT": xT,
            "wq": w_layout(w_q, cols),
            "wk": w_layout(w_k, cols),
            "wv": np.ascontiguousarray(wv_c.reshape(NKT, P, DC).transpose(1, 0, 2)).astype(BF),
            "wo": np.ascontiguousarray(w_o[cols, :]).astype(BF),
        })

    res = None
    for attempt in range(3):
        try:
            res = run_bass_kernel_spmd(nc, in_maps, list(range(NCORES)))
            break
        except Exception:
            if attempt == 2:
                raise
            import time
            time.sleep(2.0)
    acc = res.results[0]["out"].astype(np.float32)
    for c in range(1, NCORES):
        acc = acc + res.results[c]["out"].astype(np.float32)
    acc = acc + b_o[None, :]
    return acc.reshape(B, S, D)
